# revision 16
# baseline (speedup 1.0000x reference)
"""AttentionBlock (GroupNorm -> qkv -> 8-head attention -> proj -> residual)
distributed over 8 TRN2 NeuronCores.

Sharding: core = (batch b, head-half hh): each core handles 1 of 4 batches and
4 of 8 heads.  qkv_w rows / proj_w cols are split by head (tensor parallel);
the two proj partials per batch are summed on the host (no collectives).

Reference semantics (B=4, C=512, T=2048, 8 groups, 8 heads, head dim 64):
  h   = GroupNorm(x) * gn_w + gn_b
  qkv = qkv_w @ h + qkv_b            # [3C, T]; head h owns rows [192h:192h+192)
  q,k,v per head; S = (q/8^.5 . k/8^.5); A = softmax(S); a = A @ v
  out = x + proj_w @ a + proj_b

Device dataflow per core (all matmuls bf16 with f32 PSUM accumulation):
  - GroupNorm stats: per-partition sum/sumsq (DVE), group-reduce via selector
    matmul, rstd = exp(-0.5*ln(var+eps)) (stays in the natural_log_exp ACT
    table set used by the softmax exp), broadcast back via selector matmul.
  - qkv: h[C,T] x wqkT -> q,k [256,T]; v computed directly transposed
    (lhsT=h-slices) as vT[s, c] with a ones-column appended per head.
  - attention per (head, t-chunk): S^T tiles [s=128, t=512] via lhsT=k,
    exp on ScalarE with fused *0.125 scale (no max subtraction: scores are
    ~N(0,1), exp is safe in f32), PV with lhsT=vT_aug (M=65) so the softmax
    denominator accumulates in PSUM row 64 for free.
  - normalize: reciprocal of denom (DVE), broadcast across partitions via a
    K=1 ones matmul, multiply; proj + residual (xres carries x + proj_b on
    even cores, zeros on odd cores so the pair sums correctly on host).

qkv_b is all-zeros per the problem spec; the q/k bias add is implemented (it
is free in the PSUM->SBUF copy) but the v bias is not (it would need a
separate pass; it is exactly zero here).
"""

import os
import numpy as np
from contextlib import ExitStack
SKIP = set(os.environ.get('K_SKIP','').split(','))
PHASE = os.environ.get('K_PHASE','')

import concourse.bass as bass
import concourse.bacc as bacc
import concourse.tile as tile
from concourse import mybir
from concourse import bass_utils
from ml_dtypes import bfloat16

B, C, T, H, GROUPS = 4, 512, 2048, 8, 8
CH = C // H          # 64 head dim
HL = H // 2          # 4 local heads per core
CL = HL * CH         # 256 local channels
EPS = 1e-5
P = 128
TC_ = 512            # t-chunk
NT = T // TC_        # 4
NS = T // P          # 16 s-tiles
NCT = C // P         # 4 channel tiles
F32 = mybir.dt.float32
BF16 = mybir.dt.bfloat16
AX = mybir.AxisListType
OP = mybir.AluOpType
AF = mybir.ActivationFunctionType

_cached = {}


def _emit(ctx: ExitStack, tc: tile.TileContext, io: dict):
    nc = tc.nc
    x, xres, wqkT, wvT, wpT = io["x"], io["xres"], io["wqkT"], io["wvT"], io["wpT"]
    selT, sel8, gnw, gnb, qkb, out = (
        io["selT"], io["sel8"], io["gnw"], io["gnb"], io["qkb"], io["out"])

    # ---- pools ----
    persist = ctx.enter_context(tc.tile_pool(name="persist", bufs=1))
    xpool = ctx.enter_context(tc.tile_pool(name="xpool", bufs=1))
    hpool = ctx.enter_context(tc.tile_pool(name="hpool", bufs=1))
    qkpool = ctx.enter_context(tc.tile_pool(name="qkpool", bufs=1))
    vtpool = ctx.enter_context(tc.tile_pool(name="vtpool", bufs=1))
    scratch = ctx.enter_context(tc.tile_pool(name="scratch", bufs=2))
    small = ctx.enter_context(tc.tile_pool(name="small", bufs=2))
    exppool = ctx.enter_context(tc.tile_pool(name="exppool", bufs=3))
    apool_sb = ctx.enter_context(tc.tile_pool(name="apool_sb", bufs=2))
    opool = ctx.enter_context(tc.tile_pool(name="opool", bufs=3))
    xrpool = ctx.enter_context(tc.tile_pool(name="xrpool", bufs=3))
    # PSUM: 2*2 + 2*1 + 2*1 = 8 banks
    psum_qk = ctx.enter_context(tc.tile_pool(name="psum_qk", bufs=2, space="PSUM"))
    psum_a = ctx.enter_context(tc.tile_pool(name="psum_a", bufs=2, space="PSUM"))
    psum_mm = ctx.enter_context(tc.tile_pool(name="psum_mm", bufs=2, space="PSUM"))

    # ---- constants / weights to SBUF ----
    selT_sb = [persist.tile([P, GROUPS], F32, tag=f"selT{t}", name=f"selT{t}") for t in range(NCT)]
    for t in range(NCT):
        nc.sync.dma_start(out=selT_sb[t], in_=selT[P * t:P * (t + 1), :])
    sel8_sb = persist.tile([GROUPS, C], F32, tag="sel8", name="sel8")
    nc.sync.dma_start(out=sel8_sb, in_=sel8)
    gnw_sb = [persist.tile([P, 1], F32, tag=f"gnw{t}", name=f"gnw{t}") for t in range(NCT)]
    gnb_sb = [persist.tile([P, 1], F32, tag=f"gnb{t}", name=f"gnb{t}") for t in range(NCT)]
    qkb_sb = [persist.tile([P, 1], F32, tag=f"qkb{t}", name=f"qkb{t}") for t in range(NCT)]
    for t in range(NCT):
        nc.sync.dma_start(out=gnw_sb[t], in_=gnw[P * t:P * (t + 1), :])
        nc.sync.dma_start(out=gnb_sb[t], in_=gnb[P * t:P * (t + 1), :])
        nc.sync.dma_start(out=qkb_sb[t], in_=qkb[P * t:P * (t + 1), :])
    wqkT_sb = [persist.tile([P, 2 * CL], BF16, tag=f"wqkT{t}", name=f"wqkT{t}") for t in range(NCT)]
    wvT_sb = [persist.tile([P, CL], BF16, tag=f"wvT{t}", name=f"wvT{t}") for t in range(NCT)]
    for t in range(NCT):
        nc.sync.dma_start(out=wqkT_sb[t], in_=wqkT[P * t:P * (t + 1), :])
        nc.sync.dma_start(out=wvT_sb[t], in_=wvT[P * t:P * (t + 1), :])
    wpT_sb = [persist.tile([P, C], BF16, tag=f"wpT{t}", name=f"wpT{t}") for t in range(2)]
    for t in range(2):
        nc.sync.dma_start(out=wpT_sb[t], in_=wpT[P * t:P * (t + 1), :])
    ones_sb = persist.tile([1, CH], F32, tag="ones", name="ones")
    nc.vector.memset(ones_sb, 1.0)
    eps_sb = persist.tile([GROUPS, 1], F32, tag="eps", name="eps")
    nc.vector.memset(eps_sb, EPS)

    # ---- PE warmup: keep TensorE busy during the initial DMA/stats phase so
    # the HAM clock gate reaches K=8/8 (2.4 GHz) before the real matmuls ----
    NWARM = 56
    warm_w = persist.tile([P, P], BF16, tag="warm_w", name="warm_w")
    warm_r = persist.tile([P, TC_], BF16, tag="warm_r", name="warm_r")
    nc.vector.memset(warm_w, 0.001)
    nc.vector.memset(warm_r, 0.001)
    wu = psum_qk.tile([P, 2 * TC_], F32, tag="pq", name="wu")
    for i in range(NWARM):
        nc.tensor.matmul(wu[:, 0:TC_], lhsT=warm_w, rhs=warm_r,
                         start=(i == 0), stop=(i == NWARM - 1),
                         skip_group_check=True)
    warm_sink = small.tile([1, 1], F32, tag="wsink", name="wsink")
    nc.vector.tensor_copy(out=warm_sink, in_=wu[0:1, 0:1])

    # ---- phase A: load x, GroupNorm stats (bn_stats per partition, then a
    # selector matmul averages the 64 partitions of each group) ----
    BNF = nc.vector.BN_STATS_FMAX
    NBN = T // BNF
    x_sb = [xpool.tile([P, T], F32, tag=f"x{t}", name=f"x{t}") for t in range(NCT)]
    s12 = [small.tile([P, 2], F32, tag=f"s12_{t}", name=f"s12_{t}") for t in range(NCT)]
    ps_st = psum_mm.tile([P, TC_], F32, tag="mm", name="mm")
    dma_eng = [nc.sync, nc.gpsimd]
    for t in range(NCT):
        dma_eng[t % 2].dma_start(out=x_sb[t], in_=x[P * t:P * (t + 1), :])
    for t in range(NCT):
        nc.vector.tensor_reduce(out=s12[t][:, 0:1], in_=x_sb[t], axis=AX.X, op=OP.add)
        sq = scratch.tile([P, T], F32, tag="sq", name="sq")
        # note: tensor_tensor_reduce (custom-ISA DVE op) crashes this runtime
        nc.vector.tensor_mul(sq, x_sb[t], x_sb[t])
        nc.vector.tensor_reduce(out=s12[t][:, 1:2], in_=sq, axis=AX.X, op=OP.add)
        nc.tensor.matmul(ps_st[0:GROUPS, 0:2], lhsT=selT_sb[t], rhs=s12[t],
                         start=(t == 0), stop=(t == NCT - 1))

    NELT = float(CH * T)  # 131072 elements per group
    stats = small.tile([GROUPS, 2], F32, tag="stats", name="stats")
    nc.vector.tensor_scalar_mul(stats, ps_st[0:GROUPS, 0:2], 1.0 / NELT)
    var = small.tile([GROUPS, 1], F32, tag="var", name="var")
    nc.vector.tensor_mul(var, stats[:, 0:1], stats[:, 0:1])
    nc.vector.tensor_sub(var, stats[:, 1:2], var)
    # rstd = exp(-0.5 * ln(var + eps)) : stays in the natural_log_exp table set
    lnv = small.tile([GROUPS, 1], F32, tag="lnv", name="lnv")
    nc.scalar.activation(lnv, var, AF.Ln, bias=eps_sb[:, 0:1])
    mr = small.tile([GROUPS, 2], F32, tag="mr", name="mr")
    nc.vector.tensor_copy(out=mr[:, 0:1], in_=stats[:, 0:1])
    nc.scalar.activation(mr[:, 1:2], lnv, AF.Exp, scale=-0.5)

    # broadcast (mean, rstd) to channels; h = x*scale + bias  (bf16)
    h_sb = [hpool.tile([P, T], BF16, tag=f"h{t}", name=f"h{t}") for t in range(NCT)]
    for t in range(NCT):
        ps_bc = psum_mm.tile([P, TC_], F32, tag="mm", name="mm")
        nc.tensor.matmul(ps_bc[:, 0:2], lhsT=sel8_sb[:, P * t:P * (t + 1)], rhs=mr,
                         start=True, stop=True)
        sc = small.tile([P, 1], F32, tag=f"sc{t}", name=f"sc{t}")
        bi = small.tile([P, 1], F32, tag=f"bi{t}", name=f"bi{t}")
        nc.vector.tensor_mul(sc, ps_bc[:, 1:2], gnw_sb[t])
        nc.vector.tensor_mul(bi, ps_bc[:, 0:1], sc)
        nc.vector.tensor_sub(bi, gnb_sb[t], bi)
        nc.vector.tensor_scalar(out=h_sb[t], in0=x_sb[t], scalar1=sc, scalar2=bi,
                                op0=OP.mult, op1=OP.add)


    def bail_out():
        for t in range(NCT):
            nc.sync.dma_start(out=out[P * t:P * (t + 1), :], in_=x_sb[t])

    if PHASE == 'A':
        bail_out()
        return
    # ---- phase B: qkv ----
    # q,k: [256, T] each as 2 o-tiles; layout col m*128+p of wqkT
    qk_sb = [qkpool.tile([P, T], BF16, tag=f"qk{m}", name=f"qk{m}") for m in range(4)]

    def qkv_psum(idx):
        return psum_mm.tile([P, TC_], F32, tag="mm", name="mm")

    for m in range(4):
        for j in range(NT):
            pmm = qkv_psum(m * NT + j)
            for c4 in range(NCT):
                nc.tensor.matmul(pmm, lhsT=wqkT_sb[c4][:, P * m:P * (m + 1)],
                                 rhs=h_sb[c4][:, TC_ * j:TC_ * (j + 1)],
                                 start=(c4 == 0), stop=(c4 == NCT - 1))
            nc.vector.tensor_scalar_add(out=qk_sb[m][:, TC_ * j:TC_ * (j + 1)],
                                        in0=pmm, scalar1=qkb_sb[m])
    # vT_aug tiles: [128 s, 4 heads x (64 ch + ones)]
    vt_sb = []
    for i in range(NS):
        vt = vtpool.tile([P, HL * (CH + 1)], BF16, tag=f"vt{i}", name=f"vt{i}")
        vt3 = vt.rearrange("p (h c) -> p h c", c=CH + 1)
        nc.vector.memset(vt3[:, :, CH:CH + 1], 1.0)
        pv = qkv_psum(i)[:, 0:CL]
        for c4 in range(NCT):
            nc.tensor.matmul(pv, lhsT=h_sb[c4][:, P * i:P * (i + 1)],
                             rhs=wvT_sb[c4], start=(c4 == 0), stop=(c4 == NCT - 1))
        nc.vector.tensor_copy(out=vt3[:, :, 0:CH],
                              in_=pv.rearrange("p (h c) -> p h c", c=CH))
        vt_sb.append(vt)

    if PHASE == 'B':
        bail_out()
        return
    # ---- phase C+D: attention + proj, per t-chunk ----
    # Normalize (PE broadcast matmul + DVE finalize) and proj are emitted one
    # unit late ("pending") so the in-order PE queue never stalls waiting on
    # the reciprocal chain of the current head.
    NHALF = NS // 2  # 8 super-tiles of 2 s-tiles
    pending = []

    def emit_normalize(pa, r_sb, a_dst):
        def go():
            pr = psum_mm.tile([P, TC_], F32, tag="mm", name="mm")
            nc.tensor.matmul(pr[0:CH, :], lhsT=ones_sb, rhs=r_sb,
                             start=True, stop=True, skip_group_check=True)
            # only one DVE input may be PSUM: stage a_raw through SBUF
            a_tmp = small.tile([CH, TC_], F32, tag="a_tmp", name="a_tmp")
            nc.vector.tensor_copy(out=a_tmp, in_=pa[0:CH, :])
            nc.vector.tensor_mul(out=a_dst, in0=a_tmp, in1=pr[0:CH, :])
        return go

    def emit_proj(j, a_js):
        def go():
            for m in range(NCT):
                pmm = psum_mm.tile([P, TC_], F32, tag="mm", name="mm")
                for cp in range(2):
                    nc.tensor.matmul(pmm, lhsT=wpT_sb[cp][:, P * m:P * (m + 1)],
                                     rhs=a_js[cp], start=(cp == 0), stop=(cp == 1),
                                     skip_group_check=True)
                xr = xrpool.tile([P, TC_], F32, tag="xr", name="xr")
                nc.sync.dma_start(out=xr,
                                  in_=xres[P * m:P * (m + 1), TC_ * j:TC_ * (j + 1)])
                osb = opool.tile([P, TC_], F32, tag="o", name="o")
                nc.vector.tensor_add(osb, pmm, xr)
                nc.sync.dma_start(out=out[P * m:P * (m + 1), TC_ * j:TC_ * (j + 1)],
                                  in_=osb)
        return go

    # Heads are processed in PAIRS: head A lives at partitions 0:64 of the
    # q/k tiles, head B at 64:128.  The two QK matmuls of a pair use disjoint
    # PE row groups (tile_position (0,0) / (64,0)) and run CONCURRENTLY,
    # halving QK wall time; one [128,1024] exp covers both heads' S^T tiles.
    for j in range(NT):
        a_js = [apool_sb.tile([P, TC_], BF16, tag=f"apair{pair}", name=f"apair{pair}")
                for pair in range(2)]
        for pairP in range(2):
            hdA, hdB = 2 * pairP, 2 * pairP + 1
            q_t = qk_sb[pairP]
            k_t = qk_sb[2 + pairP]
            pa = [psum_a.tile([P, TC_], F32, tag="pa", name="pa") for _ in range(2)]

            def pv_mms(e, i):
                for hh, hd in ((0, hdA), (1, hdB)):
                    nc.tensor.matmul(
                        pa[hh][0:CH + 1, :],
                        lhsT=vt_sb[i][:, (CH + 1) * hd:(CH + 1) * (hd + 1)],
                        rhs=e[:, TC_ * hh:TC_ * (hh + 1)],
                        start=(i == 0), stop=(i == NS - 1),
                        skip_group_check=True)

            prev = None
            for i in range(NS):
                pq = psum_qk.tile([P, 2 * TC_], F32, tag="pq", name="pq")
                for hh, off in ((0, 0), (1, CH)):
                    nc.tensor.matmul(pq[:, TC_ * hh:TC_ * (hh + 1)],
                                     lhsT=k_t[off:off + CH, P * i:P * (i + 1)],
                                     rhs=q_t[off:off + CH, TC_ * j:TC_ * (j + 1)],
                                     tile_position=(off, 0),
                                     start=True, stop=True, skip_group_check=True)
                e = exppool.tile([P, 2 * TC_], BF16, tag="exp", name="exp")
                nc.scalar.activation(e, pq, AF.Exp, scale=0.125)
                if prev is not None:
                    pv_mms(*prev)
                if i == 1:
                    # late ops (previous unit's proj), while this unit's
                    # QK/PV stream keeps the PE fed
                    for go in pending:
                        go()
                    pending.clear()
                prev = (e, i)
            pv_mms(*prev)

            # softmax denominator reciprocal on ScalarE: 1/d = exp(-ln d)
            # (same ACT table set as the softmax exp; DVE reciprocal costs
            # ~4us/call and stalled the PE via the broadcast matmul)
            for hh, hd in ((0, hdA), (1, hdB)):
                lnd = small.tile([1, TC_], F32, tag="lnd", name="lnd")
                nc.scalar.activation(lnd, pa[hh][CH:CH + 1, :], AF.Ln)
                r_sb = small.tile([1, TC_], F32, tag="r", name="r")
                nc.scalar.activation(r_sb, lnd, AF.Exp, scale=-1.0)
                off = CH * (hd % 2)
                emit_normalize(pa[hh], r_sb, a_js[hd // 2][off:off + CH, :])()
        pending.append(emit_proj(j, a_js))
    for go in pending:
        go()
    pending.clear()


def build_nc():
    if "nc" in _cached:
        return _cached["nc"]
    nc = bacc.Bacc("TRN2", target_bir_lowering=False, debug=False, num_devices=8)
    io = {
        "x": nc.dram_tensor("x", [C, T], F32, kind="ExternalInput").ap(),
        "xres": nc.dram_tensor("xres", [C, T], F32, kind="ExternalInput").ap(),
        "wqkT": nc.dram_tensor("wqkT", [C, 2 * CL], BF16, kind="ExternalInput").ap(),
        "wvT": nc.dram_tensor("wvT", [C, CL], BF16, kind="ExternalInput").ap(),
        "wpT": nc.dram_tensor("wpT", [CL, C], BF16, kind="ExternalInput").ap(),
        "selT": nc.dram_tensor("selT", [C, GROUPS], F32, kind="ExternalInput").ap(),
        "sel8": nc.dram_tensor("sel8", [GROUPS, C], F32, kind="ExternalInput").ap(),
        "gnw": nc.dram_tensor("gnw", [C, 1], F32, kind="ExternalInput").ap(),
        "gnb": nc.dram_tensor("gnb", [C, 1], F32, kind="ExternalInput").ap(),
        "qkb": nc.dram_tensor("qkb", [2 * CL, 1], F32, kind="ExternalInput").ap(),
        "out": nc.dram_tensor("out", [C, T], F32, kind="ExternalOutput").ap(),
    }
    with tile.TileContext(nc) as tc:
        with ExitStack() as ctx:
            _emit(ctx, tc, io)
    nc.compile()
    _cached["nc"] = nc
    return nc


def make_in_maps(x, gn_w, gn_b, qkv_w, qkv_b, proj_w, proj_b):
    x = np.asarray(x, np.float32)
    qkv_w = np.asarray(qkv_w, np.float32)
    qkv_b = np.asarray(qkv_b, np.float32)
    proj_w = np.asarray(proj_w, np.float32)
    proj_b = np.asarray(proj_b, np.float32)
    gn_w = np.asarray(gn_w, np.float32)
    gn_b = np.asarray(gn_b, np.float32)

    cg = np.arange(C) // CH
    selT = (cg[:, None] == np.arange(GROUPS)[None, :]).astype(np.float32)
    sel8 = np.ascontiguousarray(selT.T)

    in_maps = []
    for core in range(8):
        b, hh = core // 2, core % 2
        heads = [hh * HL + i for i in range(HL)]
        # reference layout: qkv row block for head h is [192h : 192h+192) = q|k|v
        q_rows = np.concatenate([np.arange(192 * h, 192 * h + 64) for h in heads])
        k_rows = q_rows + 64
        v_rows = q_rows + 128
        wqk = np.concatenate([qkv_w[q_rows], qkv_w[k_rows]], 0)       # [512, 512]
        wqkT_ = np.ascontiguousarray(wqk.T).astype(bfloat16)          # [C, 512]
        wvT_ = np.ascontiguousarray(qkv_w[v_rows].T).astype(bfloat16)  # [C, 256]
        wpT_ = np.ascontiguousarray(
            proj_w[:, CL * hh:CL * (hh + 1)].T).astype(bfloat16)      # [256, C]
        qkb_ = np.concatenate([qkv_b[q_rows], qkv_b[k_rows]])[:, None].copy()
        xb = np.ascontiguousarray(x[b])
        if hh == 0:
            xres = (xb + proj_b[:, None]).astype(np.float32)
        else:
            xres = np.zeros_like(xb)
        in_maps.append({
            "x": xb, "xres": xres, "wqkT": wqkT_, "wvT": wvT_, "wpT": wpT_,
            "selT": selT, "sel8": sel8,
            "gnw": np.ascontiguousarray(gn_w[:, None]),
            "gnb": np.ascontiguousarray(gn_b[:, None]),
            "qkb": np.ascontiguousarray(qkb_),
        })
    return in_maps


def run(inputs, trace=False, **kw):
    nc = build_nc()
    in_maps = make_in_maps(**inputs)
    res = bass_utils.run_bass_kernel_spmd(
        nc, in_maps, core_ids=list(range(8)), trace=trace, **kw)
    outs = [res.results[i]["out"] for i in range(8)]
    out = np.stack([outs[2 * b] + outs[2 * b + 1] for b in range(B)]).astype(np.float32)
    return out, res


def kernel(**inputs):
    out, _ = run(inputs, trace=False)
    return out


# revision 19
# speedup vs baseline: 1.2593x; 1.2593x over previous
"""AttentionBlock (GroupNorm -> qkv -> 8-head attention -> proj -> residual)
distributed over 8 TRN2 NeuronCores.

Sharding: core = (batch b, head-half hh): each core handles 1 of 4 batches and
4 of 8 heads.  qkv_w rows / proj_w cols are split by head (tensor parallel);
the two proj partials per batch are summed on the host (no collectives).

Reference semantics (B=4, C=512, T=2048, 8 groups, 8 heads, head dim 64):
  h   = GroupNorm(x) * gn_w + gn_b
  qkv = qkv_w @ h + qkv_b            # [3C, T]; head h owns rows [192h:192h+192)
  q,k,v per head; S = (q/8^.5 . k/8^.5); A = softmax(S); a = A @ v
  out = x + proj_w @ a + proj_b

Device dataflow per core (all matmuls bf16 with f32 PSUM accumulation):
  - GroupNorm stats: per-partition sum/sumsq (DVE), group-reduce via selector
    matmul, rstd = exp(-0.5*ln(var+eps)) (stays in the natural_log_exp ACT
    table set used by the softmax exp), broadcast back via selector matmul.
  - qkv: h[C,T] x wqkT -> q,k [256,T]; v computed directly transposed
    (lhsT=h-slices) as vT[s, c] with a ones-column appended per head.
  - attention per (head, t-chunk): S^T tiles [s=128, t=512] via lhsT=k,
    exp on ScalarE with fused *0.125 scale (no max subtraction: scores are
    ~N(0,1), exp is safe in f32), PV with lhsT=vT_aug (M=65) so the softmax
    denominator accumulates in PSUM row 64 for free.
  - normalize: reciprocal of denom (DVE), broadcast across partitions via a
    K=1 ones matmul, multiply; proj + residual (xres carries x + proj_b on
    even cores, zeros on odd cores so the pair sums correctly on host).

qkv_b is all-zeros per the problem spec; the q/k bias add is implemented (it
is free in the PSUM->SBUF copy) but the v bias is not (it would need a
separate pass; it is exactly zero here).
"""

import os
import numpy as np
from contextlib import ExitStack
SKIP = set(os.environ.get('K_SKIP','').split(','))
PHASE = os.environ.get('K_PHASE','')

import concourse.bass as bass
import concourse.bacc as bacc
import concourse.tile as tile
from concourse import mybir
from concourse import bass_utils
from ml_dtypes import bfloat16

B, C, T, H, GROUPS = 4, 512, 2048, 8, 8
CH = C // H          # 64 head dim
HL = H // 2          # 4 local heads per core
CL = HL * CH         # 256 local channels
EPS = 1e-5
P = 128
TC_ = 512            # t-chunk
NT = T // TC_        # 4
NS = T // P          # 16 s-tiles
NCT = C // P         # 4 channel tiles
F32 = mybir.dt.float32
BF16 = mybir.dt.bfloat16
AX = mybir.AxisListType
OP = mybir.AluOpType
AF = mybir.ActivationFunctionType

_cached = {}


def _emit(ctx: ExitStack, tc: tile.TileContext, io: dict):
    nc = tc.nc
    x, xres, wqkT, wvT, wpT = io["x"], io["xres"], io["wqkT"], io["wvT"], io["wpT"]
    selT, sel8, gnw, gnb, qkb, out = (
        io["selT"], io["sel8"], io["gnw"], io["gnb"], io["qkb"], io["out"])

    # ---- pools ----
    persist = ctx.enter_context(tc.tile_pool(name="persist", bufs=1))
    xpool = ctx.enter_context(tc.tile_pool(name="xpool", bufs=1))
    hpool = ctx.enter_context(tc.tile_pool(name="hpool", bufs=1))
    qkpool = ctx.enter_context(tc.tile_pool(name="qkpool", bufs=1))
    vtpool = ctx.enter_context(tc.tile_pool(name="vtpool", bufs=1))
    scratch = ctx.enter_context(tc.tile_pool(name="scratch", bufs=2))
    small = ctx.enter_context(tc.tile_pool(name="small", bufs=2))
    exppool = ctx.enter_context(tc.tile_pool(name="exppool", bufs=3))
    apool_sb = ctx.enter_context(tc.tile_pool(name="apool_sb", bufs=2))
    opool = ctx.enter_context(tc.tile_pool(name="opool", bufs=3))
    xrpool = ctx.enter_context(tc.tile_pool(name="xrpool", bufs=3))
    # PSUM: 2*2 + 2*1 + 2*1 = 8 banks
    psum_qk = ctx.enter_context(tc.tile_pool(name="psum_qk", bufs=2, space="PSUM"))
    psum_a = ctx.enter_context(tc.tile_pool(name="psum_a", bufs=2, space="PSUM"))
    psum_mm = ctx.enter_context(tc.tile_pool(name="psum_mm", bufs=2, space="PSUM"))

    # ---- constants / weights to SBUF ----
    selT_sb = [persist.tile([P, GROUPS], F32, tag=f"selT{t}", name=f"selT{t}") for t in range(NCT)]
    for t in range(NCT):
        nc.sync.dma_start(out=selT_sb[t], in_=selT[P * t:P * (t + 1), :])
    sel8_sb = persist.tile([GROUPS, C], F32, tag="sel8", name="sel8")
    nc.sync.dma_start(out=sel8_sb, in_=sel8)
    gnw_sb = [persist.tile([P, 1], F32, tag=f"gnw{t}", name=f"gnw{t}") for t in range(NCT)]
    gnb_sb = [persist.tile([P, 1], F32, tag=f"gnb{t}", name=f"gnb{t}") for t in range(NCT)]
    qkb_sb = [persist.tile([P, 1], F32, tag=f"qkb{t}", name=f"qkb{t}") for t in range(NCT)]
    for t in range(NCT):
        nc.sync.dma_start(out=gnw_sb[t], in_=gnw[P * t:P * (t + 1), :])
        nc.sync.dma_start(out=gnb_sb[t], in_=gnb[P * t:P * (t + 1), :])
        nc.sync.dma_start(out=qkb_sb[t], in_=qkb[P * t:P * (t + 1), :])
    wqkT_sb = [persist.tile([P, 2 * CL], BF16, tag=f"wqkT{t}", name=f"wqkT{t}") for t in range(NCT)]
    wvT_sb = [persist.tile([P, CL], BF16, tag=f"wvT{t}", name=f"wvT{t}") for t in range(NCT)]
    for t in range(NCT):
        nc.sync.dma_start(out=wqkT_sb[t], in_=wqkT[P * t:P * (t + 1), :])
        nc.sync.dma_start(out=wvT_sb[t], in_=wvT[P * t:P * (t + 1), :])
    wpT_sb = [persist.tile([P, C], BF16, tag=f"wpT{t}", name=f"wpT{t}") for t in range(2)]
    for t in range(2):
        nc.sync.dma_start(out=wpT_sb[t], in_=wpT[P * t:P * (t + 1), :])
    ones_sb = persist.tile([1, CH], F32, tag="ones", name="ones")
    nc.vector.memset(ones_sb, 1.0)
    eps_sb = persist.tile([GROUPS, 1], F32, tag="eps", name="eps")
    nc.vector.memset(eps_sb, EPS)

    # ---- PE warmup: keep TensorE busy during the initial DMA/stats phase so
    # the HAM clock gate reaches K=8/8 (2.4 GHz) before the real matmuls ----
    NWARM = 56
    warm_w = persist.tile([P, P], BF16, tag="warm_w", name="warm_w")
    warm_r = persist.tile([P, TC_], BF16, tag="warm_r", name="warm_r")
    nc.vector.memset(warm_w, 0.001)
    nc.vector.memset(warm_r, 0.001)
    wu = psum_qk.tile([P, 2 * TC_], F32, tag="pq", name="wu")
    for i in range(NWARM):
        nc.tensor.matmul(wu[:, 0:TC_], lhsT=warm_w, rhs=warm_r,
                         start=(i == 0), stop=(i == NWARM - 1),
                         skip_group_check=True)
    warm_sink = small.tile([1, 1], F32, tag="wsink", name="wsink")
    nc.vector.tensor_copy(out=warm_sink, in_=wu[0:1, 0:1])

    # ---- phase A: load x, GroupNorm stats (bn_stats per partition, then a
    # selector matmul averages the 64 partitions of each group) ----
    BNF = nc.vector.BN_STATS_FMAX
    NBN = T // BNF
    x_sb = [xpool.tile([P, T], F32, tag=f"x{t}", name=f"x{t}") for t in range(NCT)]
    s12 = [small.tile([P, 2], F32, tag=f"s12_{t}", name=f"s12_{t}") for t in range(NCT)]
    ps_st = psum_mm.tile([P, TC_], F32, tag="mm", name="mm")
    dma_eng = [nc.sync, nc.gpsimd]
    for t in range(NCT):
        dma_eng[t % 2].dma_start(out=x_sb[t], in_=x[P * t:P * (t + 1), :])
    for t in range(NCT):
        # s1 on DVE, s2 on ScalarE (Square + free-axis accumulate) in parallel
        nc.vector.tensor_reduce(out=s12[t][:, 0:1], in_=x_sb[t], axis=AX.X, op=OP.add)
        sq = scratch.tile([P, T], BF16, tag="sq", name="sq")
        nc.scalar.activation(sq, x_sb[t], AF.Square, accum_out=s12[t][:, 1:2])
        nc.tensor.matmul(ps_st[0:GROUPS, 0:2], lhsT=selT_sb[t], rhs=s12[t],
                         start=(t == 0), stop=(t == NCT - 1))

    NELT = float(CH * T)  # 131072 elements per group
    stats = small.tile([GROUPS, 2], F32, tag="stats", name="stats")
    nc.vector.tensor_scalar_mul(stats, ps_st[0:GROUPS, 0:2], 1.0 / NELT)
    var = small.tile([GROUPS, 1], F32, tag="var", name="var")
    nc.vector.tensor_mul(var, stats[:, 0:1], stats[:, 0:1])
    nc.vector.tensor_sub(var, stats[:, 1:2], var)
    # rstd = exp(-0.5 * ln(var + eps)) : stays in the natural_log_exp table set
    lnv = small.tile([GROUPS, 1], F32, tag="lnv", name="lnv")
    nc.scalar.activation(lnv, var, AF.Ln, bias=eps_sb[:, 0:1])
    mr = small.tile([GROUPS, 2], F32, tag="mr", name="mr")
    nc.vector.tensor_copy(out=mr[:, 0:1], in_=stats[:, 0:1])
    nc.scalar.activation(mr[:, 1:2], lnv, AF.Exp, scale=-0.5)

    # broadcast (mean, rstd) to channels; h = x*scale + bias  (bf16)
    h_sb = [hpool.tile([P, T], BF16, tag=f"h{t}", name=f"h{t}") for t in range(NCT)]
    for t in range(NCT):
        ps_bc = psum_mm.tile([P, TC_], F32, tag="mm", name="mm")
        nc.tensor.matmul(ps_bc[:, 0:2], lhsT=sel8_sb[:, P * t:P * (t + 1)], rhs=mr,
                         start=True, stop=True)
        sc = small.tile([P, 1], F32, tag=f"sc{t}", name=f"sc{t}")
        bi = small.tile([P, 1], F32, tag=f"bi{t}", name=f"bi{t}")
        nc.vector.tensor_mul(sc, ps_bc[:, 1:2], gnw_sb[t])
        nc.vector.tensor_mul(bi, ps_bc[:, 0:1], sc)
        nc.vector.tensor_sub(bi, gnb_sb[t], bi)
        nc.vector.tensor_scalar(out=h_sb[t], in0=x_sb[t], scalar1=sc, scalar2=bi,
                                op0=OP.mult, op1=OP.add)


    def bail_out():
        for t in range(NCT):
            nc.sync.dma_start(out=out[P * t:P * (t + 1), :], in_=x_sb[t])

    if PHASE == 'A':
        bail_out()
        return
    # ---- phase B: qkv ----
    # q,k: [256, T] each as 2 o-tiles; layout col m*128+p of wqkT
    qk_sb = [qkpool.tile([P, T], BF16, tag=f"qk{m}", name=f"qk{m}") for m in range(4)]

    def qkv_psum(idx):
        return psum_mm.tile([P, TC_], F32, tag="mm", name="mm")

    for m in range(4):
        for j in range(NT):
            pmm = qkv_psum(m * NT + j)
            for c4 in range(NCT):
                nc.tensor.matmul(pmm, lhsT=wqkT_sb[c4][:, P * m:P * (m + 1)],
                                 rhs=h_sb[c4][:, TC_ * j:TC_ * (j + 1)],
                                 start=(c4 == 0), stop=(c4 == NCT - 1))
            nc.vector.tensor_scalar_add(out=qk_sb[m][:, TC_ * j:TC_ * (j + 1)],
                                        in0=pmm, scalar1=qkb_sb[m])
    # vT_aug tiles: [128 s, 4 heads x (64 ch + ones)]
    vt_sb = []
    for i in range(NS):
        vt = vtpool.tile([P, HL * (CH + 1)], BF16, tag=f"vt{i}", name=f"vt{i}")
        vt3 = vt.rearrange("p (h c) -> p h c", c=CH + 1)
        nc.vector.memset(vt3[:, :, CH:CH + 1], 1.0)
        pv = qkv_psum(i)[:, 0:CL]
        for c4 in range(NCT):
            nc.tensor.matmul(pv, lhsT=h_sb[c4][:, P * i:P * (i + 1)],
                             rhs=wvT_sb[c4], start=(c4 == 0), stop=(c4 == NCT - 1))
        nc.vector.tensor_copy(out=vt3[:, :, 0:CH],
                              in_=pv.rearrange("p (h c) -> p h c", c=CH))
        vt_sb.append(vt)

    if PHASE == 'B':
        bail_out()
        return
    # ---- phase C+D: attention + proj, per t-chunk ----
    # Normalize (PE broadcast matmul + DVE finalize) and proj are emitted one
    # unit late ("pending") so the in-order PE queue never stalls waiting on
    # the reciprocal chain of the current head.
    NHALF = NS // 2  # 8 super-tiles of 2 s-tiles
    pend_norm = []
    pend_proj = []

    def emit_normalize(pa, r_sb, a_dst):
        def go():
            pr = psum_mm.tile([P, TC_], F32, tag="mm", name="mm")
            nc.tensor.matmul(pr[0:CH, :], lhsT=ones_sb, rhs=r_sb,
                             start=True, stop=True, skip_group_check=True)
            # only one DVE input may be PSUM: stage a_raw through SBUF
            a_tmp = small.tile([CH, TC_], F32, tag="a_tmp", name="a_tmp")
            nc.vector.tensor_copy(out=a_tmp, in_=pa[0:CH, :])
            nc.vector.tensor_mul(out=a_dst, in0=a_tmp, in1=pr[0:CH, :])
        return go

    def prefetch_xres(j):
        xrs = []
        for m in range(NCT):
            xr = xrpool.tile([P, TC_], F32, tag="xr", name="xr")
            nc.sync.dma_start(out=xr,
                              in_=xres[P * m:P * (m + 1), TC_ * j:TC_ * (j + 1)])
            xrs.append(xr)
        return xrs

    def emit_proj(j, a_js, xrs):
        def go():
            for m in range(NCT):
                pmm = psum_mm.tile([P, TC_], F32, tag="mm", name="mm")
                for cp in range(2):
                    nc.tensor.matmul(pmm, lhsT=wpT_sb[cp][:, P * m:P * (m + 1)],
                                     rhs=a_js[cp], start=(cp == 0), stop=(cp == 1),
                                     skip_group_check=True)
                osb = opool.tile([P, TC_], F32, tag="o", name="o")
                nc.vector.tensor_add(osb, pmm, xrs[m])
                nc.sync.dma_start(out=out[P * m:P * (m + 1), TC_ * j:TC_ * (j + 1)],
                                  in_=osb)
        return go

    # Heads are processed in PAIRS: head A lives at partitions 0:64 of the
    # q/k tiles, head B at 64:128.  The two QK matmuls of a pair use disjoint
    # PE row groups (tile_position (0,0) / (64,0)) and run CONCURRENTLY,
    # halving QK wall time; one [128,1024] exp covers both heads' S^T tiles.
    for j in range(NT):
        a_js = [apool_sb.tile([P, TC_], BF16, tag=f"apair{pair}", name=f"apair{pair}")
                for pair in range(2)]
        xrs = prefetch_xres(j)
        for pairP in range(2):
            hdA, hdB = 2 * pairP, 2 * pairP + 1
            q_t = qk_sb[pairP]
            k_t = qk_sb[2 + pairP]
            pa = [psum_a.tile([P, TC_], F32, tag="pa", name="pa") for _ in range(2)]

            def pv_mms(e, i):
                for hh, hd in ((0, hdA), (1, hdB)):
                    nc.tensor.matmul(
                        pa[hh][0:CH + 1, :],
                        lhsT=vt_sb[i][:, (CH + 1) * hd:(CH + 1) * (hd + 1)],
                        rhs=e[:, TC_ * hh:TC_ * (hh + 1)],
                        start=(i == 0), stop=(i == NS - 1),
                        skip_group_check=True)

            prev = None
            for i in range(NS):
                pq = psum_qk.tile([P, 2 * TC_], F32, tag="pq", name="pq")
                for hh, off in ((0, 0), (1, CH)):
                    nc.tensor.matmul(pq[:, TC_ * hh:TC_ * (hh + 1)],
                                     lhsT=k_t[off:off + CH, P * i:P * (i + 1)],
                                     rhs=q_t[off:off + CH, TC_ * j:TC_ * (j + 1)],
                                     tile_position=(off, 0),
                                     start=True, stop=True, skip_group_check=True)
                e = exppool.tile([P, 2 * TC_], BF16, tag="exp", name="exp")
                nc.scalar.activation(e, pq, AF.Exp, scale=0.125)
                if prev is not None:
                    pv_mms(*prev)
                if i == 1:
                    # late ops (previous unit), while this unit's QK/PV
                    # stream keeps the PE fed
                    for go in pend_norm + pend_proj:
                        go()
                    pend_norm.clear()
                    pend_proj.clear()
                prev = (e, i)
            pv_mms(*prev)

            # softmax denominator reciprocal on ScalarE: 1/d = exp(-ln d)
            # (same ACT table set as the softmax exp)
            for hh, hd in ((0, hdA), (1, hdB)):
                lnd = small.tile([1, TC_], F32, tag="lnd", name="lnd")
                nc.scalar.activation(lnd, pa[hh][CH:CH + 1, :], AF.Ln)
                r_sb = small.tile([1, TC_], F32, tag="r", name="r")
                nc.scalar.activation(r_sb, lnd, AF.Exp, scale=-1.0)
                off = CH * (hd % 2)
                pend_norm.append(
                    emit_normalize(pa[hh], r_sb, a_js[hd // 2][off:off + CH, :]))
        pend_proj.append(emit_proj(j, a_js, xrs))
    for go in pend_norm + pend_proj:
        go()
    pend_norm.clear()
    pend_proj.clear()


def build_nc():
    if "nc" in _cached:
        return _cached["nc"]
    nc = bacc.Bacc("TRN2", target_bir_lowering=False, debug=False, num_devices=8)
    io = {
        "x": nc.dram_tensor("x", [C, T], F32, kind="ExternalInput").ap(),
        "xres": nc.dram_tensor("xres", [C, T], F32, kind="ExternalInput").ap(),
        "wqkT": nc.dram_tensor("wqkT", [C, 2 * CL], BF16, kind="ExternalInput").ap(),
        "wvT": nc.dram_tensor("wvT", [C, CL], BF16, kind="ExternalInput").ap(),
        "wpT": nc.dram_tensor("wpT", [CL, C], BF16, kind="ExternalInput").ap(),
        "selT": nc.dram_tensor("selT", [C, GROUPS], F32, kind="ExternalInput").ap(),
        "sel8": nc.dram_tensor("sel8", [GROUPS, C], F32, kind="ExternalInput").ap(),
        "gnw": nc.dram_tensor("gnw", [C, 1], F32, kind="ExternalInput").ap(),
        "gnb": nc.dram_tensor("gnb", [C, 1], F32, kind="ExternalInput").ap(),
        "qkb": nc.dram_tensor("qkb", [2 * CL, 1], F32, kind="ExternalInput").ap(),
        "out": nc.dram_tensor("out", [C, T], F32, kind="ExternalOutput").ap(),
    }
    with tile.TileContext(nc) as tc:
        with ExitStack() as ctx:
            _emit(ctx, tc, io)
    nc.compile()
    _cached["nc"] = nc
    return nc


def make_in_maps(x, gn_w, gn_b, qkv_w, qkv_b, proj_w, proj_b):
    x = np.asarray(x, np.float32)
    qkv_w = np.asarray(qkv_w, np.float32)
    qkv_b = np.asarray(qkv_b, np.float32)
    proj_w = np.asarray(proj_w, np.float32)
    proj_b = np.asarray(proj_b, np.float32)
    gn_w = np.asarray(gn_w, np.float32)
    gn_b = np.asarray(gn_b, np.float32)

    cg = np.arange(C) // CH
    selT = (cg[:, None] == np.arange(GROUPS)[None, :]).astype(np.float32)
    sel8 = np.ascontiguousarray(selT.T)

    in_maps = []
    for core in range(8):
        b, hh = core // 2, core % 2
        heads = [hh * HL + i for i in range(HL)]
        # reference layout: qkv row block for head h is [192h : 192h+192) = q|k|v
        q_rows = np.concatenate([np.arange(192 * h, 192 * h + 64) for h in heads])
        k_rows = q_rows + 64
        v_rows = q_rows + 128
        wqk = np.concatenate([qkv_w[q_rows], qkv_w[k_rows]], 0)       # [512, 512]
        wqkT_ = np.ascontiguousarray(wqk.T).astype(bfloat16)          # [C, 512]
        wvT_ = np.ascontiguousarray(qkv_w[v_rows].T).astype(bfloat16)  # [C, 256]
        wpT_ = np.ascontiguousarray(
            proj_w[:, CL * hh:CL * (hh + 1)].T).astype(bfloat16)      # [256, C]
        qkb_ = np.concatenate([qkv_b[q_rows], qkv_b[k_rows]])[:, None].copy()
        xb = np.ascontiguousarray(x[b])
        if hh == 0:
            xres = (xb + proj_b[:, None]).astype(np.float32)
        else:
            xres = np.zeros_like(xb)
        in_maps.append({
            "x": xb, "xres": xres, "wqkT": wqkT_, "wvT": wvT_, "wpT": wpT_,
            "selT": selT, "sel8": sel8,
            "gnw": np.ascontiguousarray(gn_w[:, None]),
            "gnb": np.ascontiguousarray(gn_b[:, None]),
            "qkb": np.ascontiguousarray(qkb_),
        })
    return in_maps


def run(inputs, trace=False, **kw):
    nc = build_nc()
    in_maps = make_in_maps(**inputs)
    res = bass_utils.run_bass_kernel_spmd(
        nc, in_maps, core_ids=list(range(8)), trace=trace, **kw)
    outs = [res.results[i]["out"] for i in range(8)]
    out = np.stack([outs[2 * b] + outs[2 * b + 1] for b in range(B)]).astype(np.float32)
    return out, res


def kernel(**inputs):
    out, _ = run(inputs, trace=False)
    return out
